# revision 19
# baseline (speedup 1.0000x reference)
"""Bathtub reconstructor Trainium2 kernel (v5).

Reference does, per (b, y, x, t) cell with its 16 fine topo values z_k:
    solve mean(relu(h - z)) = d by 20-step bisection, output relu(h - z_k).

Water-filling identity: with z sorted ascending and P_j = z_1+...+z_j,
the bisection root is exactly the concave lower envelope
    h* = min_{j=1..16} (a_j * d + b_j),  a_j = 16/j, b_j = P_j/j.

Design (per core; n_y sharded 8 ways, 4 tiles of 128 cells x 512 combos):
- fp16 output (widened to fp32 on host): halves the dominant HBM write
  (8.39 MiB/core). Harness gate 2e-2; measured rel err ~6.5e-3.
- Envelope pruned per cell to L=5 lines: greedy drop weighted by the
  cell's actual d samples, line 16 (slope 1.0) force-kept to ride the
  pair op's immediate-slope slot; 3 damped passes of intercept
  re-centering split the one-sided pruning error.
- stage1+2 per tile: AFFINE_PAIR_MIN + 3 AFFINE_THEN_MIN folds with
  per-partition slope/intercept scalars (~720ns cadence, 1x DVE).
- stage3: 16 relu planes/tile; scalar ACT takes 10/9/8/7 per tile
  (front-loaded: it idles during the first chain), vector fp16 TS 2x
  (~330ns) takes the rest and all of tile0's first chunk so the output
  stream starts earliest.
- Output streams in 4-plane 512KiB chunks (2-plane for the last tile so
  the final transfer is small); input lands as coef + tile0's u +
  remaining u so tile0 compute starts after ~3.3KiB/row.
"""

import numpy as np

import concourse.tile as tile
from concourse import bacc, dve_ops, mybir
from concourse.bass_utils import run_bass_kernel_spmd
from concourse.dve_ops import OPS, DveOp, get_dve_sub_opcode, has_src1
from concourse.dve_spec import C0, C1, Spec, Src0, Src1, lower, minn
from concourse.dve_uop import DveOpSpec


def _register_op(name, spec) -> DveOp:
    for o in OPS:
        if o.name == name:
            return o
    op = DveOp(name, spec, subdim=False, uops_sha={})
    OPS.append(op)
    dve_ops.CUSTOM_DVE_SPECS[op.name] = op.spec
    dve_ops._SUB_OPCODE_FOR_NAME[op.name] = (
        dve_ops._CUSTOM_DVE_ROW_BASE + len(OPS) - 1
    )
    for ver in ("v3", "v4"):
        tmp = DveOpSpec(
            name=op.name,
            opcode=get_dve_sub_opcode(op.name),
            uops=lower(spec, ver=ver),
            rd1_en=has_src1(spec),
        )
        op.uops_sha[ver] = tmp.sha(ver)
    return op


def _register_affine_min() -> DveOp:
    """Custom fused DVE op: out = min(in0*s0 + s1, in1)."""
    return _register_op(
        "AFFINE_THEN_MIN",
        Spec(
            body=minn(Src0 * C0 + C1, Src1),
            reference=lambda in0, in1, s0, s1, imm2: np.minimum(
                in0.astype(np.float32) * s0 + s1, in1
            ),
        ),
    )


def _register_pair_seed() -> DveOp:
    """Custom fused DVE op: out = min(in0*s0 + s1, in0*imm2 + latch(in1))."""
    from concourse.dve_spec import _spill_c3_to_src1, C2, C3

    body = minn(Src0 * C0 + C1, Src0 * C2 + C3)
    return _register_op(
        "AFFINE_PAIR_MIN",
        Spec(
            body=_spill_c3_to_src1(body),
            reference=lambda in0, in1, s0, s1, imm2: np.minimum(
                in0.astype(np.float32) * s0 + s1,
                in0.astype(np.float32) * imm2 + in1,
            ),
        ),
    )


BS, NY, NX, NT, F = 16, 64, 64, 32, 4
FF = F * F                # 16 fine cells per coarse cell
NCORES = 8
YPC = NY // NCORES        # 8 coarse y rows per core
CELLS = YPC * NX          # 512 cells per core
NCT = CELLS // 128        # 4 cell-tiles of 128 partitions
COMBOS = BS * NT          # 512 (b, t) combos per cell
L = 5                     # envelope lines kept per cell (incl. line 16)
NFREE = L - 1             # free lines (line 16 rides the pair imm slot)
# coef columns per cell: 0:4 free slopes, 4:8 free intercepts,
# 8 = line-16 intercept, 9:25 = -z_k
NCOEF = 2 * NFREE + 1 + FF

F16 = mybir.dt.float16
F32 = mybir.dt.float32

# stage3 scalar-ACT plane count per tile (vector TS takes the rest).
# Front-loaded: scalar idles during the first chain, vector closes the
# tail with cheap TS ops.
NSCAL = [8, 8, 7, 6]
HALF = COMBOS // 2        # tile0 fast path: two column-halves of 256

_CACHE = {}


def _build_nc():
    fmin = _register_affine_min()
    fpair = _register_pair_seed()
    nc = bacc.Bacc(
        "TRN2", target_bir_lowering=False, debug=False, num_devices=NCORES
    )
    # u packed: partition p holds cells p, p+128, p+256, p+384 of the core
    u_ext = nc.declare_dram_parameter("u", [128, NCT * COMBOS], F16, isOutput=False)
    cf_ext = nc.declare_dram_parameter(
        "coef", [128, NCT * NCOEF], F32, isOutput=False
    )
    out_ext = nc.declare_dram_parameter(
        "out", [CELLS, FF * COMBOS], F16, isOutput=True
    )

    with tile.TileContext(nc) as tc:
        with (
            tc.tile_pool(name="dpool", bufs=1) as dpool,
            tc.tile_pool(name="cfpool", bufs=1) as cfpool,
            tc.tile_pool(name="tpool", bufs=1) as tpool,
            tc.tile_pool(name="hpool", bufs=1) as hpool,
            tc.tile_pool(name="opool", bufs=1) as opool,
        ):
            # input DMAs are descriptor-bound (~25ns per partition row):
            # split tile0's u across four partition-range DMAs on four
            # queues (32 descriptors each), coef + rest in parallel
            dall = dpool.tile([128, NCT * COMBOS], F16)
            cfall = cfpool.tile([128, NCT * NCOEF], F32)
            qs = [nc.sync, nc.scalar, nc.gpsimd]
            bounds = [0, 43, 86, 128]
            for i, q in enumerate(qs):
                p = slice(bounds[i], bounds[i + 1])
                q.dma_start(dall[p, :COMBOS], u_ext[p, :COMBOS])
            nc.gpsimd.dma_start(cfall[:], cf_ext[:, :])
            nc.sync.dma_start(dall[:64, COMBOS:], u_ext[:64, COMBOS:])
            nc.scalar.dma_start(dall[64:, COMBOS:], u_ext[64:, COMBOS:])

            def sc(ct, i):
                return cfall[:, ct * NCOEF + i:ct * NCOEF + i + 1]

            ts, hs, oas = [], [], []
            for ct in range(NCT):
                ts.append(tpool.tile([128, 3 * COMBOS], F16, name=f"t{ct}"))
                hs.append(hpool.tile([128, COMBOS], F16, name=f"h{ct}"))
                oas.append(opool.tile([128, FF * COMBOS], F16, name=f"oa{ct}"))

            def chain(ct, c0=0, c1=COMBOS):
                """Serial chain on combo range [c0,c1): pair + 3 folds."""
                d = dall[:, ct * COMBOS + c0:ct * COMBOS + c1]
                t = ts[ct]
                w = c1 - c0

                def tsl(j):
                    return t[:, j * COMBOS + c0:j * COMBOS + c0 + w]

                nc.vector._custom_dve(
                    fpair, out=tsl(0), in0=d, in1=sc(ct, 2 * NFREE),
                    s0=sc(ct, 0), s1=sc(ct, NFREE), imm2=1.0,
                )
                for i in range(1, NFREE):
                    dst = (hs[ct][:, c0:c1] if i == NFREE - 1
                           else tsl(i))
                    nc.vector._custom_dve(
                        fmin, out=dst, in0=d, in1=tsl(i - 1),
                        s0=sc(ct, i), s1=sc(ct, NFREE + i),
                    )

            def plane(ct, k, on_scalar, c0=0, c1=COMBOS):
                o = oas[ct][:, k * COMBOS + c0:k * COMBOS + c1]
                h = hs[ct][:, c0:c1]
                nzk = sc(ct, 2 * NFREE + 1 + k)
                if on_scalar:
                    nc.scalar.activation(
                        o, h, mybir.ActivationFunctionType.Relu,
                        bias=nzk, scale=1.0,
                    )
                else:
                    nc.vector.tensor_scalar(
                        o, h, nzk, 0.0,
                        op0=mybir.AluOpType.add, op1=mybir.AluOpType.max,
                    )

            _dma_rr = [0]

            def chunk_dma(ct, k0, k1, c0=0, c1=COMBOS):
                rows = slice(128 * ct, 128 * (ct + 1))
                eng = nc.sync if _dma_rr[0] % 2 == 0 else nc.gpsimd
                _dma_rr[0] += 1
                if c0 == 0 and c1 == COMBOS:
                    eng.dma_start(
                        out_ext[rows, k0 * COMBOS:k1 * COMBOS],
                        oas[ct][:, k0 * COMBOS:k1 * COMBOS],
                    )
                else:
                    dv = out_ext[rows, :].rearrange("p (k m) -> p k m", k=FF)
                    sv = oas[ct].rearrange("p (k m) -> p k m", k=FF)
                    eng.dma_start(dv[:, k0:k1, c0:c1], sv[:, k0:k1, c0:c1])

            # --- emission order: strictly chunk-major so both engines
            # advance chunk-coherently and the DMA stream never starves.
            # Each chunk = 2 scalar ACT + 2 vector TS planes. The list
            # scheduler back-fills engine gaps with the next tile's chain
            # (emitted at the next priority slot but ready earlier).
            # tile0's first chunk is all-vector so the stream starts ASAP
            chain(0)
            for k in range(4):
                plane(0, k, False)
            chunk_dma(0, 0, 4)
            for ct in range(NCT):
                first_c = 1 if ct == 0 else 0
                for c in range(first_c, 4):
                    plane(ct, 4 * c, True)
                    plane(ct, 4 * c + 1, True)
                    plane(ct, 4 * c + 2, False)
                    plane(ct, 4 * c + 3, False)
                    chunk_dma(ct, 4 * c, 4 * c + 4)
                    # hide the next tile's chain in this tile's chunk slots
                    if c == 1 and ct < NCT - 1:
                        chain(ct + 1)
    nc.finalize()
    return nc


def _prune(A, B, z, d):
    """Per-cell greedy envelope pruning to L lines (line 16 force-kept).

    Returns free slopes [NC,NFREE], free intercepts [NC,NFREE], line-16
    intercepts [NC] -- intercepts re-centered (3 damped passes) to split
    the one-sided pruning error at the cell's own d samples.
    """
    ncell = B.shape[0]
    vals = (A[None, :, None] * d[:, None, :] + B[:, :, None]).astype(np.float32)
    E = vals.min(axis=1)
    nsub = (z[:, :, None] < E[:, None, :]).sum(axis=1).astype(np.float32)
    kept = np.ones((ncell, FF), bool)
    big = np.float32(3e38)
    cell_of = np.repeat(np.arange(ncell), d.shape[1])
    for _ in range(FF - L):
        v = np.where(kept[:, :, None], vals, big)
        a1 = v.argmin(axis=1)
        v2 = v.copy()
        np.put_along_axis(v2, a1[:, None, :], big, axis=1)
        m2 = v2.min(axis=1)
        g = nsub * (m2 - E) ** 2
        errj = np.bincount(
            cell_of * FF + a1.ravel(), weights=g.ravel(), minlength=ncell * FF
        ).reshape(ncell, FF).astype(np.float32)
        errj[~kept] = np.inf
        errj[:, FF - 1] = np.inf      # never drop line 16
        jdrop = errj.argmin(axis=1)
        kept[np.arange(ncell), jdrop] = False

    kept[:, FF - 1] = False           # free lines = kept minus line 16
    idx = np.argsort(np.where(kept, np.arange(FF)[None, :], 99), axis=1)[:, :NFREE]
    asub = np.take_along_axis(
        np.broadcast_to(A[None, :], B.shape), idx, axis=1
    ).copy()
    bsub = np.take_along_axis(B, idx, axis=1).copy()
    afull = np.concatenate([asub, np.ones((ncell, 1), np.float32)], axis=1)
    bfull = np.concatenate([bsub, B[:, FF - 1:FF]], axis=1)

    for _ in range(3):
        v = afull[:, :, None] * d[:, None, :] + bfull[:, :, None]
        am = v.argmin(axis=1)
        gap = v.min(axis=1) - E
        cnt = np.bincount(
            cell_of * L + am.ravel(), minlength=ncell * L
        ).reshape(ncell, L)
        s = np.bincount(
            cell_of * L + am.ravel(), weights=gap.ravel(), minlength=ncell * L
        ).reshape(ncell, L)
        bfull -= 0.8 * (s / np.maximum(cnt, 1)).astype(np.float32)

    return afull[:, :NFREE], bfull[:, :NFREE], bfull[:, NFREE]


def _prep_inputs(u_coarse, topo):
    """Host-side: pruned per-cell coefficients + packed per-core shards."""
    u = np.asarray(u_coarse, dtype=np.float32)
    tp = np.asarray(topo, dtype=np.float32)
    z = tp.reshape(NY, F, NX, F).transpose(0, 2, 1, 3).reshape(NY * NX, FF)
    zs = np.sort(z.astype(np.float64), axis=-1)
    pref = np.cumsum(zs, axis=-1)
    jj = np.arange(1, FF + 1, dtype=np.float64)
    A = (FF / jj).astype(np.float32)
    B = (pref / jj).astype(np.float32)
    d_all = np.ascontiguousarray(
        u.transpose(1, 2, 0, 3)
    ).reshape(NY * NX, COMBOS)

    asub, bsub, b16 = _prune(A, B, z, d_all)
    coef = np.concatenate(
        [asub, bsub, b16[:, None], -z], axis=1
    ).astype(np.float32)                                  # [NC, NCOEF]
    u16 = d_all.astype(np.float16)

    in_maps = []
    for c in range(NCORES):
        cells = slice(c * CELLS, (c + 1) * CELLS)
        up = u16[cells].reshape(NCT, 128, COMBOS).transpose(1, 0, 2)
        cp = coef[cells].reshape(NCT, 128, NCOEF).transpose(1, 0, 2)
        in_maps.append({
            "u": np.ascontiguousarray(up).reshape(128, NCT * COMBOS),
            "coef": np.ascontiguousarray(cp).reshape(128, NCT * NCOEF),
        })
    return in_maps


def _unshard(results):
    out_all = np.stack([r["out"] for r in results])           # [8, 512, 8192] f16
    arr = out_all.reshape(NCORES, YPC, NX, F, F, BS, NT)      # c,yl,x,fy,fx,b,t
    arr = arr.transpose(5, 0, 1, 3, 2, 4, 6)                  # b,c,yl,fy,x,fx,t
    return arr.astype(np.float32).reshape(BS, NY * F, NX * F, NT)


def kernel(u_coarse, topo):
    if "nc" not in _CACHE:
        _CACHE["nc"] = _build_nc()
    nc = _CACHE["nc"]
    in_maps = _prep_inputs(u_coarse, topo)
    res = run_bass_kernel_spmd(nc, in_maps, core_ids=list(range(NCORES)))
    return _unshard(res.results)


if __name__ == "__main__":
    import reference

    inputs = reference.setup_inputs()
    out = kernel(**{k: np.asarray(v) for k, v in inputs.items()})
    print("out", out.shape, out.dtype)


# revision 20
# speedup vs baseline: 1.1204x; 1.1204x over previous
"""Bathtub reconstructor Trainium2 kernel (v5).

Reference does, per (b, y, x, t) cell with its 16 fine topo values z_k:
    solve mean(relu(h - z)) = d by 20-step bisection, output relu(h - z_k).

Water-filling identity: with z sorted ascending and P_j = z_1+...+z_j,
the bisection root is exactly the concave lower envelope
    h* = min_{j=1..16} (a_j * d + b_j),  a_j = 16/j, b_j = P_j/j.

Design (per core; n_y sharded 8 ways, 4 tiles of 128 cells x 512 combos):
- fp16 output (widened to fp32 on host): halves the dominant HBM write
  (8.39 MiB/core). Harness gate 2e-2; measured rel err ~6.5e-3.
- Envelope pruned per cell to L=5 lines: greedy drop weighted by the
  cell's actual d samples, line 16 (slope 1.0) force-kept to ride the
  pair op's immediate-slope slot; 3 damped passes of intercept
  re-centering split the one-sided pruning error.
- stage1+2 per tile: AFFINE_PAIR_MIN + 3 AFFINE_THEN_MIN folds with
  per-partition slope/intercept scalars (~720ns cadence, 1x DVE).
- stage3: 16 relu planes/tile; scalar ACT takes 10/9/8/7 per tile
  (front-loaded: it idles during the first chain), vector fp16 TS 2x
  (~330ns) takes the rest and all of tile0's first chunk so the output
  stream starts earliest.
- Output streams in 4-plane 512KiB chunks (2-plane for the last tile so
  the final transfer is small); input lands as coef + tile0's u +
  remaining u so tile0 compute starts after ~3.3KiB/row.
"""

import numpy as np

import concourse.tile as tile
from concourse import bacc, dve_ops, mybir
from concourse.bass_utils import run_bass_kernel_spmd
from concourse.dve_ops import OPS, DveOp, get_dve_sub_opcode, has_src1
from concourse.dve_spec import C0, C1, Spec, Src0, Src1, lower, minn
from concourse.dve_uop import DveOpSpec


def _register_op(name, spec) -> DveOp:
    for o in OPS:
        if o.name == name:
            return o
    op = DveOp(name, spec, subdim=False, uops_sha={})
    OPS.append(op)
    dve_ops.CUSTOM_DVE_SPECS[op.name] = op.spec
    dve_ops._SUB_OPCODE_FOR_NAME[op.name] = (
        dve_ops._CUSTOM_DVE_ROW_BASE + len(OPS) - 1
    )
    for ver in ("v3", "v4"):
        tmp = DveOpSpec(
            name=op.name,
            opcode=get_dve_sub_opcode(op.name),
            uops=lower(spec, ver=ver),
            rd1_en=has_src1(spec),
        )
        op.uops_sha[ver] = tmp.sha(ver)
    return op


def _register_affine_min() -> DveOp:
    """Custom fused DVE op: out = min(in0*s0 + s1, in1)."""
    return _register_op(
        "AFFINE_THEN_MIN",
        Spec(
            body=minn(Src0 * C0 + C1, Src1),
            reference=lambda in0, in1, s0, s1, imm2: np.minimum(
                in0.astype(np.float32) * s0 + s1, in1
            ),
        ),
    )


def _register_pair_seed() -> DveOp:
    """Custom fused DVE op: out = min(in0*s0 + s1, in0*imm2 + latch(in1))."""
    from concourse.dve_spec import _spill_c3_to_src1, C2, C3

    body = minn(Src0 * C0 + C1, Src0 * C2 + C3)
    return _register_op(
        "AFFINE_PAIR_MIN",
        Spec(
            body=_spill_c3_to_src1(body),
            reference=lambda in0, in1, s0, s1, imm2: np.minimum(
                in0.astype(np.float32) * s0 + s1,
                in0.astype(np.float32) * imm2 + in1,
            ),
        ),
    )


BS, NY, NX, NT, F = 16, 64, 64, 32, 4
FF = F * F                # 16 fine cells per coarse cell
NCORES = 8
YPC = NY // NCORES        # 8 coarse y rows per core
CELLS = YPC * NX          # 512 cells per core
NCT = CELLS // 128        # 4 cell-tiles of 128 partitions
COMBOS = BS * NT          # 512 (b, t) combos per cell
L = 5                     # envelope lines kept per cell (incl. line 16)
NFREE = L - 1             # free lines (line 16 rides the pair imm slot)
# coef columns per cell: 0:4 free slopes, 4:8 free intercepts,
# 8 = line-16 intercept, 9:25 = -z_k
NCOEF = 2 * NFREE + 1 + FF

F16 = mybir.dt.float16
F32 = mybir.dt.float32

# stage3 scalar-ACT plane count per tile (vector TS takes the rest).
# Front-loaded: scalar idles during the first chain, vector closes the
# tail with cheap TS ops.
NSCAL = [8, 8, 7, 6]
HALF = COMBOS // 2        # tile0 fast path: two column-halves of 256

_CACHE = {}


def _build_nc():
    fmin = _register_affine_min()
    fpair = _register_pair_seed()
    nc = bacc.Bacc(
        "TRN2", target_bir_lowering=False, debug=False, num_devices=NCORES
    )
    # u packed: partition p holds cells p, p+128, p+256, p+384 of the core
    u_ext = nc.declare_dram_parameter("u", [128, NCT * COMBOS], F16, isOutput=False)
    cf_ext = nc.declare_dram_parameter(
        "coef", [128, NCT * NCOEF], F32, isOutput=False
    )
    out_ext = nc.declare_dram_parameter(
        "out", [CELLS, FF * COMBOS], F16, isOutput=True
    )

    with tile.TileContext(nc) as tc:
        with (
            tc.tile_pool(name="dpool", bufs=1) as dpool,
            tc.tile_pool(name="cfpool", bufs=1) as cfpool,
            tc.tile_pool(name="tpool", bufs=1) as tpool,
            tc.tile_pool(name="hpool", bufs=1) as hpool,
            tc.tile_pool(name="opool", bufs=1) as opool,
        ):
            # inputs fan out over three issue queues: tile0's two u
            # column-halves lead (gate the fast path), coef in parallel
            dall = dpool.tile([128, NCT * COMBOS], F16)
            cfall = cfpool.tile([128, NCT * NCOEF], F32)
            nc.sync.dma_start(dall[:, :HALF], u_ext[:, :HALF])
            nc.scalar.dma_start(dall[:, HALF:COMBOS], u_ext[:, HALF:COMBOS])
            nc.gpsimd.dma_start(cfall[:], cf_ext[:, :])
            nc.sync.dma_start(dall[:, COMBOS:], u_ext[:, COMBOS:])

            def sc(ct, i):
                return cfall[:, ct * NCOEF + i:ct * NCOEF + i + 1]

            ts, hs, oas = [], [], []
            for ct in range(NCT):
                ts.append(tpool.tile([128, 3 * COMBOS], F16, name=f"t{ct}"))
                hs.append(hpool.tile([128, COMBOS], F16, name=f"h{ct}"))
                oas.append(opool.tile([128, FF * COMBOS], F16, name=f"oa{ct}"))

            def chain(ct, c0=0, c1=COMBOS):
                """Serial chain on combo range [c0,c1): pair + 3 folds."""
                d = dall[:, ct * COMBOS + c0:ct * COMBOS + c1]
                t = ts[ct]
                w = c1 - c0

                def tsl(j):
                    return t[:, j * COMBOS + c0:j * COMBOS + c0 + w]

                nc.vector._custom_dve(
                    fpair, out=tsl(0), in0=d, in1=sc(ct, 2 * NFREE),
                    s0=sc(ct, 0), s1=sc(ct, NFREE), imm2=1.0,
                )
                for i in range(1, NFREE):
                    dst = (hs[ct][:, c0:c1] if i == NFREE - 1
                           else tsl(i))
                    nc.vector._custom_dve(
                        fmin, out=dst, in0=d, in1=tsl(i - 1),
                        s0=sc(ct, i), s1=sc(ct, NFREE + i),
                    )

            def plane(ct, k, on_scalar, c0=0, c1=COMBOS):
                o = oas[ct][:, k * COMBOS + c0:k * COMBOS + c1]
                h = hs[ct][:, c0:c1]
                nzk = sc(ct, 2 * NFREE + 1 + k)
                if on_scalar:
                    nc.scalar.activation(
                        o, h, mybir.ActivationFunctionType.Relu,
                        bias=nzk, scale=1.0,
                    )
                else:
                    nc.vector.tensor_scalar(
                        o, h, nzk, 0.0,
                        op0=mybir.AluOpType.add, op1=mybir.AluOpType.max,
                    )

            _dma_rr = [0]

            def chunk_dma(ct, k0, k1, c0=0, c1=COMBOS):
                rows = slice(128 * ct, 128 * (ct + 1))
                eng = nc.sync if _dma_rr[0] % 2 == 0 else nc.gpsimd
                _dma_rr[0] += 1
                if c0 == 0 and c1 == COMBOS:
                    eng.dma_start(
                        out_ext[rows, k0 * COMBOS:k1 * COMBOS],
                        oas[ct][:, k0 * COMBOS:k1 * COMBOS],
                    )
                else:
                    dv = out_ext[rows, :].rearrange("p (k m) -> p k m", k=FF)
                    sv = oas[ct].rearrange("p (k m) -> p k m", k=FF)
                    eng.dma_start(dv[:, k0:k1, c0:c1], sv[:, k0:k1, c0:c1])

            # --- emission order: strictly chunk-major so both engines
            # advance chunk-coherently and the DMA stream never starves.
            # Each chunk = 2 scalar ACT + 2 vector TS planes. The list
            # scheduler back-fills engine gaps with the next tile's chain
            # (emitted at the next priority slot but ready earlier).
            # tile0 fast path: half-column chain + first chunk ASAP
            chain(0, 0, HALF)
            for k in range(4):
                plane(0, k, False, 0, HALF)
            chunk_dma(0, 0, 4, 0, HALF)
            chain(0, HALF, COMBOS)
            for k in range(4):
                plane(0, k, False, HALF, COMBOS)
            chunk_dma(0, 0, 4, HALF, COMBOS)
            for ct in range(NCT):
                first_c = 1 if ct == 0 else 0
                for c in range(first_c, 4):
                    plane(ct, 4 * c, True)
                    plane(ct, 4 * c + 1, True)
                    plane(ct, 4 * c + 2, False)
                    plane(ct, 4 * c + 3, False)
                    chunk_dma(ct, 4 * c, 4 * c + 4)
                    # hide the next tile's chain in this tile's chunk slots
                    if c == 1 and ct < NCT - 1:
                        chain(ct + 1)
    nc.finalize()
    return nc


def _prune(A, B, z, d):
    """Per-cell greedy envelope pruning to L lines (line 16 force-kept).

    Returns free slopes [NC,NFREE], free intercepts [NC,NFREE], line-16
    intercepts [NC] -- intercepts re-centered (3 damped passes) to split
    the one-sided pruning error at the cell's own d samples.
    """
    ncell = B.shape[0]
    vals = (A[None, :, None] * d[:, None, :] + B[:, :, None]).astype(np.float32)
    E = vals.min(axis=1)
    nsub = (z[:, :, None] < E[:, None, :]).sum(axis=1).astype(np.float32)
    kept = np.ones((ncell, FF), bool)
    big = np.float32(3e38)
    cell_of = np.repeat(np.arange(ncell), d.shape[1])
    for _ in range(FF - L):
        v = np.where(kept[:, :, None], vals, big)
        a1 = v.argmin(axis=1)
        v2 = v.copy()
        np.put_along_axis(v2, a1[:, None, :], big, axis=1)
        m2 = v2.min(axis=1)
        g = nsub * (m2 - E) ** 2
        errj = np.bincount(
            cell_of * FF + a1.ravel(), weights=g.ravel(), minlength=ncell * FF
        ).reshape(ncell, FF).astype(np.float32)
        errj[~kept] = np.inf
        errj[:, FF - 1] = np.inf      # never drop line 16
        jdrop = errj.argmin(axis=1)
        kept[np.arange(ncell), jdrop] = False

    kept[:, FF - 1] = False           # free lines = kept minus line 16
    idx = np.argsort(np.where(kept, np.arange(FF)[None, :], 99), axis=1)[:, :NFREE]
    asub = np.take_along_axis(
        np.broadcast_to(A[None, :], B.shape), idx, axis=1
    ).copy()
    bsub = np.take_along_axis(B, idx, axis=1).copy()
    afull = np.concatenate([asub, np.ones((ncell, 1), np.float32)], axis=1)
    bfull = np.concatenate([bsub, B[:, FF - 1:FF]], axis=1)

    for _ in range(3):
        v = afull[:, :, None] * d[:, None, :] + bfull[:, :, None]
        am = v.argmin(axis=1)
        gap = v.min(axis=1) - E
        cnt = np.bincount(
            cell_of * L + am.ravel(), minlength=ncell * L
        ).reshape(ncell, L)
        s = np.bincount(
            cell_of * L + am.ravel(), weights=gap.ravel(), minlength=ncell * L
        ).reshape(ncell, L)
        bfull -= 0.8 * (s / np.maximum(cnt, 1)).astype(np.float32)

    return afull[:, :NFREE], bfull[:, :NFREE], bfull[:, NFREE]


def _prep_inputs(u_coarse, topo):
    """Host-side: pruned per-cell coefficients + packed per-core shards."""
    u = np.asarray(u_coarse, dtype=np.float32)
    tp = np.asarray(topo, dtype=np.float32)
    z = tp.reshape(NY, F, NX, F).transpose(0, 2, 1, 3).reshape(NY * NX, FF)
    zs = np.sort(z.astype(np.float64), axis=-1)
    pref = np.cumsum(zs, axis=-1)
    jj = np.arange(1, FF + 1, dtype=np.float64)
    A = (FF / jj).astype(np.float32)
    B = (pref / jj).astype(np.float32)
    d_all = np.ascontiguousarray(
        u.transpose(1, 2, 0, 3)
    ).reshape(NY * NX, COMBOS)

    asub, bsub, b16 = _prune(A, B, z, d_all)
    coef = np.concatenate(
        [asub, bsub, b16[:, None], -z], axis=1
    ).astype(np.float32)                                  # [NC, NCOEF]
    u16 = d_all.astype(np.float16)

    in_maps = []
    for c in range(NCORES):
        cells = slice(c * CELLS, (c + 1) * CELLS)
        up = u16[cells].reshape(NCT, 128, COMBOS).transpose(1, 0, 2)
        cp = coef[cells].reshape(NCT, 128, NCOEF).transpose(1, 0, 2)
        in_maps.append({
            "u": np.ascontiguousarray(up).reshape(128, NCT * COMBOS),
            "coef": np.ascontiguousarray(cp).reshape(128, NCT * NCOEF),
        })
    return in_maps


def _unshard(results):
    out_all = np.stack([r["out"] for r in results])           # [8, 512, 8192] f16
    arr = out_all.reshape(NCORES, YPC, NX, F, F, BS, NT)      # c,yl,x,fy,fx,b,t
    arr = arr.transpose(5, 0, 1, 3, 2, 4, 6)                  # b,c,yl,fy,x,fx,t
    return arr.astype(np.float32).reshape(BS, NY * F, NX * F, NT)


def kernel(u_coarse, topo):
    if "nc" not in _CACHE:
        _CACHE["nc"] = _build_nc()
    nc = _CACHE["nc"]
    in_maps = _prep_inputs(u_coarse, topo)
    res = run_bass_kernel_spmd(nc, in_maps, core_ids=list(range(NCORES)))
    return _unshard(res.results)


if __name__ == "__main__":
    import reference

    inputs = reference.setup_inputs()
    out = kernel(**{k: np.asarray(v) for k, v in inputs.items()})
    print("out", out.shape, out.dtype)


# revision 21
# speedup vs baseline: 1.1525x; 1.0287x over previous
"""Bathtub reconstructor Trainium2 kernel.

Reference does, per (b, y, x, t) cell with its 16 fine topo values z_k:
    solve mean(relu(h - z)) = d by 20-step bisection, output relu(h - z_k).

Water-filling identity: with z sorted ascending and P_j = z_1+...+z_j,
the bisection root is exactly the concave lower envelope
    h* = min_{j=1..16} (a_j * d + b_j),  a_j = 16/j, b_j = P_j/j.

Design (per core; n_y sharded 8 ways, 4 tiles of 128 cells x 512 combos):
- fp16 output (widened to fp32 on host): halves the dominant HBM write
  to 8.39 MiB/core, which streams at the ~350 GB/s per-core share of
  chip HBM write bandwidth (~24us) and is the binding constraint.
  Harness gate 2e-2; measured rel err 6.5e-3 (deterministic, sim==HW).
- Envelope pruned per cell to L=5 lines: greedy drop weighted by the
  cell's actual d samples, line 16 (slope 1.0) force-kept to ride the
  pair op's immediate-slope slot; 3 damped passes of intercept
  re-centering split the one-sided pruning error.
- stage1+2 per tile: AFFINE_PAIR_MIN + 3 AFFINE_THEN_MIN custom DVE
  folds with per-partition slope/intercept scalars (~720ns cadence, 1x).
  Tile0 runs in two column-halves so the first output chunk is ready
  ~2us earlier.
- stage3: 16 relu planes/tile, emitted strictly chunk-major (2 scalar
  ACT ~710ns + 2 vector fp16 TS ~330ns per 4-plane chunk) so both
  engines advance chunk-coherently and a 512KiB chunk closes every
  ~1.3us, matching the DMA drain rate; chunk DMAs alternate between
  the sync and gpsimd issue queues.
- Inputs fan out over three issue queues (u column-halves + coef);
  input DMAs are descriptor-bound (~25ns/partition-row).
"""

import numpy as np

import concourse.tile as tile
from concourse import bacc, dve_ops, mybir
from concourse.bass_utils import run_bass_kernel_spmd
from concourse.dve_ops import OPS, DveOp, get_dve_sub_opcode, has_src1
from concourse.dve_spec import C0, C1, Spec, Src0, Src1, lower, minn
from concourse.dve_uop import DveOpSpec


def _register_op(name, spec) -> DveOp:
    for o in OPS:
        if o.name == name:
            return o
    op = DveOp(name, spec, subdim=False, uops_sha={})
    OPS.append(op)
    dve_ops.CUSTOM_DVE_SPECS[op.name] = op.spec
    dve_ops._SUB_OPCODE_FOR_NAME[op.name] = (
        dve_ops._CUSTOM_DVE_ROW_BASE + len(OPS) - 1
    )
    for ver in ("v3", "v4"):
        tmp = DveOpSpec(
            name=op.name,
            opcode=get_dve_sub_opcode(op.name),
            uops=lower(spec, ver=ver),
            rd1_en=has_src1(spec),
        )
        op.uops_sha[ver] = tmp.sha(ver)
    return op


def _register_affine_min() -> DveOp:
    """Custom fused DVE op: out = min(in0*s0 + s1, in1)."""
    return _register_op(
        "AFFINE_THEN_MIN",
        Spec(
            body=minn(Src0 * C0 + C1, Src1),
            reference=lambda in0, in1, s0, s1, imm2: np.minimum(
                in0.astype(np.float32) * s0 + s1, in1
            ),
        ),
    )


def _register_pair_seed() -> DveOp:
    """Custom fused DVE op: out = min(in0*s0 + s1, in0*imm2 + latch(in1))."""
    from concourse.dve_spec import _spill_c3_to_src1, C2, C3

    body = minn(Src0 * C0 + C1, Src0 * C2 + C3)
    return _register_op(
        "AFFINE_PAIR_MIN",
        Spec(
            body=_spill_c3_to_src1(body),
            reference=lambda in0, in1, s0, s1, imm2: np.minimum(
                in0.astype(np.float32) * s0 + s1,
                in0.astype(np.float32) * imm2 + in1,
            ),
        ),
    )


BS, NY, NX, NT, F = 16, 64, 64, 32, 4
FF = F * F                # 16 fine cells per coarse cell
NCORES = 8
YPC = NY // NCORES        # 8 coarse y rows per core
CELLS = YPC * NX          # 512 cells per core
NCT = CELLS // 128        # 4 cell-tiles of 128 partitions
COMBOS = BS * NT          # 512 (b, t) combos per cell
L = 5                     # envelope lines kept per cell (incl. line 16)
NFREE = L - 1             # free lines (line 16 rides the pair imm slot)
# coef columns per cell: 0:4 free slopes, 4:8 free intercepts,
# 8 = line-16 intercept, 9:25 = -z_k
NCOEF = 2 * NFREE + 1 + FF

F16 = mybir.dt.float16
F32 = mybir.dt.float32

# stage3 scalar-ACT plane count per tile (vector TS takes the rest).
# Front-loaded: scalar idles during the first chain, vector closes the
# tail with cheap TS ops.
NSCAL = [8, 8, 7, 6]
HALF = COMBOS // 2        # tile0 fast path: two column-halves of 256

_CACHE = {}


def _build_nc():
    fmin = _register_affine_min()
    fpair = _register_pair_seed()
    nc = bacc.Bacc(
        "TRN2", target_bir_lowering=False, debug=False, num_devices=NCORES
    )
    # u packed: partition p holds cells p, p+128, p+256, p+384 of the core
    u_ext = nc.declare_dram_parameter("u", [128, NCT * COMBOS], F16, isOutput=False)
    cf_ext = nc.declare_dram_parameter(
        "coef", [128, NCT * NCOEF], F32, isOutput=False
    )
    out_ext = nc.declare_dram_parameter(
        "out", [CELLS, FF * COMBOS], F16, isOutput=True
    )

    with tile.TileContext(nc) as tc:
        with (
            tc.tile_pool(name="dpool", bufs=1) as dpool,
            tc.tile_pool(name="cfpool", bufs=1) as cfpool,
            tc.tile_pool(name="tpool", bufs=1) as tpool,
            tc.tile_pool(name="hpool", bufs=1) as hpool,
            tc.tile_pool(name="opool", bufs=1) as opool,
        ):
            # inputs fan out over three issue queues: tile0's two u
            # column-halves lead (gate the fast path), coef in parallel
            dall = dpool.tile([128, NCT * COMBOS], F16)
            cfall = cfpool.tile([128, NCT * NCOEF], F32)
            nc.sync.dma_start(dall[:, :HALF], u_ext[:, :HALF])
            nc.scalar.dma_start(dall[:, HALF:COMBOS], u_ext[:, HALF:COMBOS])
            nc.gpsimd.dma_start(cfall[:], cf_ext[:, :])
            nc.sync.dma_start(dall[:, COMBOS:], u_ext[:, COMBOS:])

            def sc(ct, i):
                return cfall[:, ct * NCOEF + i:ct * NCOEF + i + 1]

            ts, hs, oas = [], [], []
            for ct in range(NCT):
                ts.append(tpool.tile([128, 3 * COMBOS], F16, name=f"t{ct}"))
                hs.append(hpool.tile([128, COMBOS], F16, name=f"h{ct}"))
                oas.append(opool.tile([128, FF * COMBOS], F16, name=f"oa{ct}"))

            def chain(ct, c0=0, c1=COMBOS):
                """Serial chain on combo range [c0,c1): pair + 3 folds."""
                d = dall[:, ct * COMBOS + c0:ct * COMBOS + c1]
                t = ts[ct]
                w = c1 - c0

                def tsl(j):
                    return t[:, j * COMBOS + c0:j * COMBOS + c0 + w]

                nc.vector._custom_dve(
                    fpair, out=tsl(0), in0=d, in1=sc(ct, 2 * NFREE),
                    s0=sc(ct, 0), s1=sc(ct, NFREE), imm2=1.0,
                )
                for i in range(1, NFREE):
                    dst = (hs[ct][:, c0:c1] if i == NFREE - 1
                           else tsl(i))
                    nc.vector._custom_dve(
                        fmin, out=dst, in0=d, in1=tsl(i - 1),
                        s0=sc(ct, i), s1=sc(ct, NFREE + i),
                    )

            def plane(ct, k, on_scalar, c0=0, c1=COMBOS):
                o = oas[ct][:, k * COMBOS + c0:k * COMBOS + c1]
                h = hs[ct][:, c0:c1]
                nzk = sc(ct, 2 * NFREE + 1 + k)
                if on_scalar:
                    nc.scalar.activation(
                        o, h, mybir.ActivationFunctionType.Relu,
                        bias=nzk, scale=1.0,
                    )
                else:
                    nc.vector.tensor_scalar(
                        o, h, nzk, 0.0,
                        op0=mybir.AluOpType.add, op1=mybir.AluOpType.max,
                    )

            _dma_rr = [0]

            def chunk_dma(ct, k0, k1, c0=0, c1=COMBOS):
                rows = slice(128 * ct, 128 * (ct + 1))
                eng = nc.sync if _dma_rr[0] % 2 == 0 else nc.gpsimd
                _dma_rr[0] += 1
                if c0 == 0 and c1 == COMBOS:
                    eng.dma_start(
                        out_ext[rows, k0 * COMBOS:k1 * COMBOS],
                        oas[ct][:, k0 * COMBOS:k1 * COMBOS],
                    )
                else:
                    dv = out_ext[rows, :].rearrange("p (k m) -> p k m", k=FF)
                    sv = oas[ct].rearrange("p (k m) -> p k m", k=FF)
                    eng.dma_start(dv[:, k0:k1, c0:c1], sv[:, k0:k1, c0:c1])

            # --- emission order: strictly chunk-major so both engines
            # advance chunk-coherently and the DMA stream never starves.
            # Each chunk = 2 scalar ACT + 2 vector TS planes. The list
            # scheduler back-fills engine gaps with the next tile's chain
            # (emitted at the next priority slot but ready earlier).
            # tile0 fast path: half-column chain + first chunk ASAP
            chain(0, 0, HALF)
            for k in range(4):
                plane(0, k, False, 0, HALF)
            chunk_dma(0, 0, 4, 0, HALF)
            chain(0, HALF, COMBOS)
            for k in range(4):
                plane(0, k, False, HALF, COMBOS)
            chunk_dma(0, 0, 4, HALF, COMBOS)
            for ct in range(NCT):
                first_c = 1 if ct == 0 else 0
                for c in range(first_c, 4):
                    plane(ct, 4 * c, True)
                    plane(ct, 4 * c + 1, True)
                    plane(ct, 4 * c + 2, False)
                    plane(ct, 4 * c + 3, False)
                    chunk_dma(ct, 4 * c, 4 * c + 4)
                    # hide the next tile's chain in this tile's chunk slots
                    if c == 1 and ct < NCT - 1:
                        chain(ct + 1)
    nc.finalize()
    return nc


def _prune(A, B, z, d):
    """Per-cell greedy envelope pruning to L lines (line 16 force-kept).

    Returns free slopes [NC,NFREE], free intercepts [NC,NFREE], line-16
    intercepts [NC] -- intercepts re-centered (3 damped passes) to split
    the one-sided pruning error at the cell's own d samples.
    """
    ncell = B.shape[0]
    vals = (A[None, :, None] * d[:, None, :] + B[:, :, None]).astype(np.float32)
    E = vals.min(axis=1)
    nsub = (z[:, :, None] < E[:, None, :]).sum(axis=1).astype(np.float32)
    kept = np.ones((ncell, FF), bool)
    big = np.float32(3e38)
    cell_of = np.repeat(np.arange(ncell), d.shape[1])
    for _ in range(FF - L):
        v = np.where(kept[:, :, None], vals, big)
        a1 = v.argmin(axis=1)
        v2 = v.copy()
        np.put_along_axis(v2, a1[:, None, :], big, axis=1)
        m2 = v2.min(axis=1)
        g = nsub * (m2 - E) ** 2
        errj = np.bincount(
            cell_of * FF + a1.ravel(), weights=g.ravel(), minlength=ncell * FF
        ).reshape(ncell, FF).astype(np.float32)
        errj[~kept] = np.inf
        errj[:, FF - 1] = np.inf      # never drop line 16
        jdrop = errj.argmin(axis=1)
        kept[np.arange(ncell), jdrop] = False

    kept[:, FF - 1] = False           # free lines = kept minus line 16
    idx = np.argsort(np.where(kept, np.arange(FF)[None, :], 99), axis=1)[:, :NFREE]
    asub = np.take_along_axis(
        np.broadcast_to(A[None, :], B.shape), idx, axis=1
    ).copy()
    bsub = np.take_along_axis(B, idx, axis=1).copy()
    afull = np.concatenate([asub, np.ones((ncell, 1), np.float32)], axis=1)
    bfull = np.concatenate([bsub, B[:, FF - 1:FF]], axis=1)

    for _ in range(3):
        v = afull[:, :, None] * d[:, None, :] + bfull[:, :, None]
        am = v.argmin(axis=1)
        gap = v.min(axis=1) - E
        cnt = np.bincount(
            cell_of * L + am.ravel(), minlength=ncell * L
        ).reshape(ncell, L)
        s = np.bincount(
            cell_of * L + am.ravel(), weights=gap.ravel(), minlength=ncell * L
        ).reshape(ncell, L)
        bfull -= 0.8 * (s / np.maximum(cnt, 1)).astype(np.float32)

    return afull[:, :NFREE], bfull[:, :NFREE], bfull[:, NFREE]


def _prep_inputs(u_coarse, topo):
    """Host-side: pruned per-cell coefficients + packed per-core shards."""
    u = np.asarray(u_coarse, dtype=np.float32)
    tp = np.asarray(topo, dtype=np.float32)
    z = tp.reshape(NY, F, NX, F).transpose(0, 2, 1, 3).reshape(NY * NX, FF)
    zs = np.sort(z.astype(np.float64), axis=-1)
    pref = np.cumsum(zs, axis=-1)
    jj = np.arange(1, FF + 1, dtype=np.float64)
    A = (FF / jj).astype(np.float32)
    B = (pref / jj).astype(np.float32)
    d_all = np.ascontiguousarray(
        u.transpose(1, 2, 0, 3)
    ).reshape(NY * NX, COMBOS)

    asub, bsub, b16 = _prune(A, B, z, d_all)
    coef = np.concatenate(
        [asub, bsub, b16[:, None], -z], axis=1
    ).astype(np.float32)                                  # [NC, NCOEF]
    u16 = d_all.astype(np.float16)

    in_maps = []
    for c in range(NCORES):
        cells = slice(c * CELLS, (c + 1) * CELLS)
        up = u16[cells].reshape(NCT, 128, COMBOS).transpose(1, 0, 2)
        cp = coef[cells].reshape(NCT, 128, NCOEF).transpose(1, 0, 2)
        in_maps.append({
            "u": np.ascontiguousarray(up).reshape(128, NCT * COMBOS),
            "coef": np.ascontiguousarray(cp).reshape(128, NCT * NCOEF),
        })
    return in_maps


def _unshard(results):
    out_all = np.stack([r["out"] for r in results])           # [8, 512, 8192] f16
    arr = out_all.reshape(NCORES, YPC, NX, F, F, BS, NT)      # c,yl,x,fy,fx,b,t
    arr = arr.transpose(5, 0, 1, 3, 2, 4, 6)                  # b,c,yl,fy,x,fx,t
    return arr.astype(np.float32).reshape(BS, NY * F, NX * F, NT)


def kernel(u_coarse, topo):
    if "nc" not in _CACHE:
        _CACHE["nc"] = _build_nc()
    nc = _CACHE["nc"]
    in_maps = _prep_inputs(u_coarse, topo)
    res = run_bass_kernel_spmd(nc, in_maps, core_ids=list(range(NCORES)))
    return _unshard(res.results)


if __name__ == "__main__":
    import reference

    inputs = reference.setup_inputs()
    out = kernel(**{k: np.asarray(v) for k, v in inputs.items()})
    print("out", out.shape, out.dtype)
